# revision 1
# baseline (speedup 1.0000x reference)
"""MultiHeadLatentAttention Trainium2 kernel (8 NeuronCores, SPMD).

Sharding: core c -> (batch b = c // 4, latent group g = c % 4).
Each core owns query heads [4g, 4g+4) and latent head g for its batch:
  - q projection for its 4 heads (1/sqrt(HD) folded into the weights)
  - k, v via HOST-MERGED latent weights: k = x @ (kr_w@kl_w)^T + (kr_w@kl_b
    + kr_b) -- mathematically identical to the two-stage latent form but
    4.25x fewer FLOPs since head_dim(128) << latent_dim(512)
  - causal attention for 4 heads in transposed layout (scoresT[s_k, s_q]):
    exp on ScalarE straight out of PSUM (no max subtraction -- scores are
    O(1) by construction), structural causality (diagonal blocks masked by
    a 0/1 tril multiply on probs, fully-masked regions skipped/memset),
    softmax denominators via an all-ones [128x128] stationary matmul
    accumulated alongside AV, normalization by DVE fast-reciprocal multiply
  - partial o_proj (its 512 input dims -> full 2048 output dims), bf16 out
Host sums the 4 partial o_proj outputs per batch (fp32) and adds o_b.

All matmuls run in bf16 with fp32 PSUM accumulation; each attention unit's
tail AV/sums are deferred past the next unit's score matmuls so the PE
never stalls on the exp chain.
"""

import math

import numpy as np
import ml_dtypes

B, S, H = 2, 2048, 2048
NH, HD = 16, 128
NLH, LD = 4, 512
HPC = 4            # query heads per core
NCORES = 8
SQC = 512          # s_q chunk
NQC = S // SQC     # 4 chunks
NKT = H // 128     # 16 contraction tiles for the projections
NSB = S // 128     # 16 s_k blocks
BF16 = ml_dtypes.bfloat16

_CACHE = {}


def _build_program(repeat=1):
    import concourse.bacc as bacc
    import concourse.bass as bass
    import concourse.tile as tile
    from concourse import mybir
    from contextlib import ExitStack

    dt = mybir.dt
    AF = mybir.ActivationFunctionType

    nc = bacc.Bacc("TRN2", target_bir_lowering=False, debug=False,
                   num_devices=NCORES)

    xT = nc.declare_dram_parameter("xT", [H, S], dt.bfloat16, isOutput=False)
    qw = nc.declare_dram_parameter("qwT", [H, HPC * HD], dt.bfloat16, isOutput=False)
    kw = nc.declare_dram_parameter("kwT", [H, HD], dt.bfloat16, isOutput=False)
    vw = nc.declare_dram_parameter("vwT", [H, HD], dt.bfloat16, isOutput=False)
    ow = nc.declare_dram_parameter("owT", [HPC * HD, H], dt.bfloat16, isOutput=False)
    qb = nc.declare_dram_parameter("qb", [HPC * HD], dt.float32, isOutput=False)
    kb = nc.declare_dram_parameter("kb", [HD], dt.float32, isOutput=False)
    vb = nc.declare_dram_parameter("vb", [HD], dt.float32, isOutput=False)
    tri = nc.declare_dram_parameter("tri", [128, 128], dt.bfloat16, isOutput=False)
    outp = nc.declare_dram_parameter("out", [H, S], dt.bfloat16, isOutput=True)

    with tile.TileContext(nc) as tc, ExitStack() as ctx:
        const = ctx.enter_context(tc.tile_pool(name="const", bufs=1))
        xpool = ctx.enter_context(tc.tile_pool(name="xpool", bufs=24))
        probs_pool = ctx.enter_context(tc.tile_pool(name="probs", bufs=18))
        attn_pool = ctx.enter_context(tc.tile_pool(name="attn", bufs=8))
        small = ctx.enter_context(tc.tile_pool(name="small", bufs=8))
        psum = ctx.enter_context(tc.tile_pool(name="psum", bufs=8, space="PSUM"))

        # ---------------- constants / weights ----------------
        # first x chunk (gpsimd queues) + q weights (sync queues) land first so
        # the PE can start ASAP; k-interleaved so (qw[k], xt[k]) pairs arrive
        # in consumption order.
        qw_sb = const.tile([128, NKT, HPC * HD], dt.bfloat16, tag="qw")
        kw_sb = const.tile([128, NKT, HD], dt.bfloat16, tag="kw")
        vw_sb = const.tile([128, NKT, HD], dt.bfloat16, tag="vw")
        # startup DMA balance: HWDGE (sync) takes kw/vw + low qw k-slices,
        # SWDGE (gpsimd) takes the x chunk + high qw k-slices; phase-1 n=0
        # runs kT -> v -> q so compute starts before qw fully lands.
        for k4 in range(4):
            nc.sync.dma_start(
                out=kw_sb[:, 4 * k4:4 * (k4 + 1), :],
                in_=kw.ap()[512 * k4:512 * (k4 + 1), :]
                .rearrange("(k p) m -> p k m", p=128))
        for k4 in range(4):
            nc.sync.dma_start(
                out=vw_sb[:, 4 * k4:4 * (k4 + 1), :],
                in_=vw.ap()[512 * k4:512 * (k4 + 1), :]
                .rearrange("(k p) m -> p k m", p=128))
        xs0 = []
        for k in range(NKT):
            xt = xpool.tile([128, SQC], dt.bfloat16, tag="xt", name="xt")
            nc.gpsimd.dma_start(out=xt, in_=xT.ap()[128 * k:128 * (k + 1), 0:SQC])
            xs0.append(xt)
            eng = nc.sync if k < 6 else nc.gpsimd
            eng.dma_start(out=qw_sb[:, k, :], in_=qw.ap()[128 * k:128 * (k + 1), :])

        qb_sb = const.tile([128, HPC], dt.float32, tag="qb")
        nc.sync.dma_start(out=qb_sb, in_=qb.ap().rearrange("(m p) -> p m", p=128))
        kb_sb = const.tile([128, 1], dt.float32, tag="kb")
        nc.sync.dma_start(out=kb_sb, in_=kb.ap().rearrange("(m p) -> p m", p=128))

        vb_ap = vb.ap()
        vb_bc = const.tile([128, HD], dt.float32, tag="vbb")
        nc.sync.dma_start(
            out=vb_bc,
            in_=bass.AP(tensor=vb_ap.tensor, offset=vb_ap.offset,
                        ap=[[0, 128]] + list(vb_ap.ap)),
        )
        tri_sb = const.tile([128, 128], dt.bfloat16, tag="tri")
        nc.sync.dma_start(out=tri_sb, in_=tri.ap())
        ones_sb = const.tile([128, 128], dt.bfloat16, tag="ones")
        nc.vector.memset(ones_sb, 1.0)

        # persistent activations
        qT_sb = [const.tile([128, S], dt.bfloat16, tag=f"qT{h}", name=f"qT{h}")
                 for h in range(HPC)]
        kT_sb = const.tile([128, S], dt.bfloat16, tag="kT")
        v_sb = const.tile([128, NSB, HD], dt.bfloat16, tag="v")

        def _emit_body(first):
            # ---------------- phase 1: projections ----------------
            for n in range(NQC):
                if first and n == 0:
                    xs = xs0
                else:
                    xs = []
                    for k in range(NKT):
                        xt = xpool.tile([128, SQC], dt.bfloat16, tag="xt", name="xt")
                        eng = nc.sync if k % 2 == 0 else nc.gpsimd
                        eng.dma_start(
                            out=xt,
                            in_=xT.ap()[128 * k:128 * (k + 1), SQC * n:SQC * (n + 1)])
                        xs.append(xt)

                # kT[:, n-chunk] = (kr_w @ kl_w) @ x^T  (host-merged weight)
                ps = psum.tile([128, SQC], dt.float32, tag="bank")
                for k in range(NKT):
                    nc.tensor.matmul(ps, lhsT=kw_sb[:, k, :], rhs=xs[k],
                                     start=(k == 0), stop=(k == NKT - 1))
                nc.scalar.activation(out=kT_sb[:, SQC * n:SQC * (n + 1)], in_=ps,
                                     func=AF.Identity, bias=kb_sb[:, 0:1])

                # v natural [s, hd] = x @ (vr_w @ vl_w)^T  (host-merged weight)
                for jj in range(4):
                    ps = psum.tile([128, SQC], dt.float32, tag="bank")
                    for k in range(NKT):
                        nc.tensor.matmul(ps[:, :HD],
                                         lhsT=xs[k][:, 128 * jj:128 * (jj + 1)],
                                         rhs=vw_sb[:, k, :],
                                         start=(k == 0), stop=(k == NKT - 1))
                    nc.vector.tensor_add(out=v_sb[:, 4 * n + jj, :], in0=ps[:, :HD],
                                         in1=vb_bc)

                for h in range(HPC):
                    ps = psum.tile([128, SQC], dt.float32, tag="bank")
                    for k in range(NKT):
                        nc.tensor.matmul(ps, lhsT=qw_sb[:, k, 128 * h:128 * (h + 1)],
                                         rhs=xs[k], start=(k == 0), stop=(k == NKT - 1))
                    nc.scalar.activation(out=qT_sb[h][:, SQC * n:SQC * (n + 1)], in_=ps,
                                         func=AF.Identity, bias=qb_sb[:, h:h + 1])

                if n == 0:
                    # o_proj weights: first consumed ~45us in (o_proj of iq=0)
                    ow_sb = const.tile([128, 4, H], dt.bfloat16, tag="ow")
                    nc.sync.dma_start(
                        out=ow_sb, in_=ow.ap().rearrange("(k p) m -> p k m", p=128))

                # ---- attention + o_proj for iq = n (kT/v/qT chunks 0..n ready)
                # Each unit's tail AV/sums + normalize are deferred until after
                # the NEXT unit's scores, so the PE never waits on the
                # exp->mask chain of the current unit.
                iq = n
                attn_tiles = []
                prev_tail = None

                def make_unit(h):
                    J = 4 * iq + 4
                    state = {"av": None, "sum": None}

                    def emit_av(j, pt):
                        if state["av"] is None:
                            state["av"] = psum.tile([128, SQC], dt.float32,
                                                    tag="bank", name="ps_av")
                            state["sum"] = psum.tile([128, SQC], dt.float32,
                                                     tag="bank", name="ps_sum")
                        nc.tensor.matmul(state["av"], lhsT=v_sb[:, j, :], rhs=pt,
                                         start=(j == 0), stop=(j == J - 1))
                        nc.tensor.matmul(state["sum"], lhsT=ones_sb, rhs=pt,
                                         start=(j == 0), stop=(j == J - 1))

                    pending = []

                    def emit_scores():
                        nonlocal pending
                        for j in range(J):
                            ps_s = psum.tile([128, SQC], dt.float32, tag="bank",
                                             name="ps_s")
                            nc.tensor.matmul(ps_s,
                                             lhsT=kT_sb[:, 128 * j:128 * (j + 1)],
                                             rhs=qT_sb[h][:, SQC * iq:SQC * (iq + 1)],
                                             start=True, stop=True)
                            pt = probs_pool.tile([128, SQC], dt.bfloat16, tag="pt",
                                                 name="pt")
                            d = j - 4 * iq
                            if d < 0:
                                nc.scalar.activation(out=pt, in_=ps_s, func=AF.Exp)
                            else:
                                if d > 0:
                                    nc.vector.memset(pt[:, :128 * d], 0.0)
                                nc.scalar.activation(out=pt[:, 128 * d:],
                                                     in_=ps_s[:, 128 * d:],
                                                     func=AF.Exp)
                                nc.vector.tensor_mul(
                                    out=pt[:, 128 * d:128 * (d + 1)],
                                    in0=pt[:, 128 * d:128 * (d + 1)], in1=tri_sb)
                            pending.append((j, pt))
                            if len(pending) > 3:
                                emit_av(*pending.pop(0))

                    def emit_tail():
                        for p in pending:
                            emit_av(*p)
                        recip = small.tile([128, SQC], dt.float32, tag="recip",
                                           name="recip")
                        nc.vector.reciprocal_approx_fast(out=recip, in_=state["sum"])
                        at = attn_pool.tile([128, SQC], dt.bfloat16, tag="at",
                                            name="at")
                        nc.vector.tensor_mul(out=at, in0=state["av"], in1=recip)
                        attn_tiles.append(at)

                    return emit_scores, emit_tail

                for h in range(HPC):
                    emit_scores, emit_tail = make_unit(h)
                    emit_scores()
                    if prev_tail is not None:
                        prev_tail()
                    prev_tail = emit_tail
                prev_tail()

                for m in range(NSB):
                    ps_o = psum.tile([128, SQC], dt.float32, tag="bank")
                    for h in range(HPC):
                        nc.tensor.matmul(ps_o, lhsT=ow_sb[:, h, 128 * m:128 * (m + 1)],
                                         rhs=attn_tiles[h], start=(h == 0), stop=(h == 3))
                    o_sb = small.tile([128, SQC], dt.bfloat16, tag="osb")
                    nc.vector.tensor_copy(out=o_sb, in_=ps_o)
                    nc.sync.dma_start(
                        out=outp.ap()[128 * m:128 * (m + 1), SQC * iq:SQC * (iq + 1)],
                        in_=o_sb)

        for _rep in range(repeat):
            _emit_body(first=(_rep == 0))

    nc.compile()
    return nc


def _get_nc(repeat=1):
    key = f"nc{repeat}"
    if key not in _CACHE:
        _CACHE[key] = _build_program(repeat)
    return _CACHE[key]


def _make_in_maps(hidden_states, attention_mask, q_w, q_b, kl_w, kl_b, vl_w, vl_b,
                  kr_w, kr_b, vr_w, vr_b, o_w):
    scale = 1.0 / math.sqrt(HD)
    tri01 = (np.asarray(attention_mask[0, 0, :128, :128]) == 0).T.astype(BF16)
    kr_f = np.asarray(kr_w, np.float32)
    vr_f = np.asarray(vr_w, np.float32)
    in_maps = []
    for c in range(NCORES):
        b, g = divmod(c, NLH)
        sl = slice(LD * g, LD * (g + 1))
        xTc = np.ascontiguousarray(np.asarray(hidden_states[b], np.float32).T
                                   ).astype(BF16)
        # merged latent->head weights: k = x @ (kr_w @ kl_w)^T + (kr_w@kl_b + kr_b)
        kw_eff = kr_f @ np.asarray(kl_w[sl], np.float32)
        vw_eff = vr_f @ np.asarray(vl_w[sl], np.float32)
        kb_eff = kr_f @ np.asarray(kl_b[sl], np.float32) + np.asarray(kr_b, np.float32)
        vb_eff = vr_f @ np.asarray(vl_b[sl], np.float32) + np.asarray(vr_b, np.float32)
        in_maps.append({
            "xT": xTc,
            "qwT": np.ascontiguousarray(
                (np.asarray(q_w[sl], np.float32) * scale).T).astype(BF16),
            "kwT": np.ascontiguousarray(kw_eff.T).astype(BF16),
            "vwT": np.ascontiguousarray(vw_eff.T).astype(BF16),
            "owT": np.ascontiguousarray(np.asarray(o_w, np.float32)[:, sl].T
                                        ).astype(BF16),
            "qb": (np.asarray(q_b[sl], np.float32) * scale),
            "kb": kb_eff,
            "vb": vb_eff,
            "tri": tri01,
        })
    return in_maps


def _gather(results, o_b):
    o_b = np.asarray(o_b, np.float32)
    outs = []
    for b in range(B):
        acc = np.zeros((H, S), np.float32)
        for g in range(NLH):
            acc += results[b * NLH + g]["out"].astype(np.float32)
        outs.append(acc.T + o_b[None, :])
    return np.stack(outs).astype(np.float32)


def kernel(hidden_states, position_ids, attention_mask, q_w, q_b, kl_w, kl_b,
           vl_w, vl_b, kr_w, kr_b, vr_w, vr_b, o_w, o_b):
    from concourse.bass_utils import run_bass_kernel_spmd

    nc = _get_nc()
    in_maps = _make_in_maps(hidden_states, attention_mask, q_w, q_b, kl_w, kl_b,
                            vl_w, vl_b, kr_w, kr_b, vr_w, vr_b, o_w)
    res = run_bass_kernel_spmd(nc, in_maps, core_ids=list(range(NCORES)))
    return _gather(res.results, o_b)



# revision 6
# speedup vs baseline: 1.4351x; 1.4351x over previous
"""MultiHeadLatentAttention Trainium2 kernel (8 NeuronCores, SPMD).

Sharding: core c -> (batch b = c // 4, latent group g = c % 4).
Each core owns query heads [4g, 4g+4) and latent head g for its batch:
  - q projection for its 4 heads (1/sqrt(HD) folded into the weights)
  - k, v via HOST-MERGED latent weights: k = x @ (kr_w@kl_w)^T + (kr_w@kl_b
    + kr_b) -- mathematically identical to the two-stage latent form but
    4.25x fewer FLOPs since head_dim(128) << latent_dim(512)
  - causal attention for 4 heads in transposed layout (scoresT[s_k, s_q])
  - partial o_proj (its 512 input dims -> full 2048 output dims), fp16 out
Host sums the 4 partial o_proj outputs per batch (fp32) and adds o_b.

v3 design (vs the 292us bf16 baseline):
  - fp16 everywhere (same PE rate as bf16, 8x finer mantissa)
  - softmax denominators OFF the PE: probs accumulate on DVE into a fp16
    acc (even/odd j-blocks in the two halves of a [128,1024] tile), then
    1-2 ones-matmuls per (head, chunk) reduce across partitions
  - score blocks processed as PAIRS in [128,1024] double-bank PSUM tiles:
    one exp + one acc-add per pair (halves Act/DVE instruction count);
    the 4 causal-diagonal blocks are column-restricted and packed into two
    contiguous pair tiles ([512+384] and [256+128] cols)
  - AV matmuls column-restricted to the causal-valid region
  - fine-grained emission interleave: between score pairs the PE queue gets
    "filler" matmuls (o_proj pairs of the previous chunk, the previous
    unit's AV/den) so the in-order PE never waits on the exp chain;
    PSUM = 2 score-pair slots + 1 av/den slot + 1 o-pair slot
  - o_proj weights + x chunk 0 / O(3) / attention tail pipelined across
    body boundaries (steady-state repeat bodies have no phase bubbles);
    v is double-buffered across bodies to keep the carried tail correct

All matmuls run in fp16 with fp32 PSUM accumulation.
"""

import math

import numpy as np

B, S, H = 2, 2048, 2048
NH, HD = 16, 128
NLH, LD = 4, 512
HPC = 4            # query heads per core
NCORES = 8
SQC = 512          # s_q chunk
NQC = S // SQC     # 4 chunks
NKT = H // 128     # 16 contraction tiles for the projections
FP16 = np.float16

_CACHE = {}


def _build_program(repeat=1):
    import concourse.bacc as bacc
    import concourse.bass as bass
    import concourse.tile as tile
    from concourse import mybir
    from contextlib import ExitStack

    dt = mybir.dt
    AF = mybir.ActivationFunctionType

    nc = bacc.Bacc("TRN2", target_bir_lowering=False, debug=False,
                   num_devices=NCORES)

    xT = nc.declare_dram_parameter("xT", [H, S], dt.float16, isOutput=False)
    qw = nc.declare_dram_parameter("qwT", [H, HPC * HD], dt.float16, isOutput=False)
    kw = nc.declare_dram_parameter("kwT", [H, HD], dt.float16, isOutput=False)
    vw = nc.declare_dram_parameter("vwT", [H, HD], dt.float16, isOutput=False)
    ow = nc.declare_dram_parameter("owT", [HPC * HD, H], dt.float16, isOutput=False)
    qb = nc.declare_dram_parameter("qb", [HPC * HD], dt.float32, isOutput=False)
    kb = nc.declare_dram_parameter("kb", [HD], dt.float32, isOutput=False)
    vb = nc.declare_dram_parameter("vb", [HD], dt.float32, isOutput=False)
    tri = nc.declare_dram_parameter("tri", [128, 128], dt.float16, isOutput=False)
    outp = nc.declare_dram_parameter("out", [H, S], dt.float16, isOutput=True)

    with tile.TileContext(nc) as tc, ExitStack() as ctx:
        const = ctx.enter_context(tc.tile_pool(name="const", bufs=1))
        xpool = ctx.enter_context(tc.tile_pool(name="xpool", bufs=34))
        probs_pool = ctx.enter_context(tc.tile_pool(name="probs", bufs=20))
        acc_pool = ctx.enter_context(tc.tile_pool(name="accp", bufs=6))
        attn_pool = ctx.enter_context(tc.tile_pool(name="attn", bufs=14))
        small = ctx.enter_context(tc.tile_pool(name="small", bufs=5))
        psum_sc = ctx.enter_context(tc.tile_pool(name="psc", bufs=2, space="PSUM"))
        psum_tl = ctx.enter_context(tc.tile_pool(name="ptl", bufs=1, space="PSUM"))
        psum_o = ctx.enter_context(tc.tile_pool(name="po", bufs=1, space="PSUM"))

        # ---------------- constants / weights ----------------
        qw_sb = const.tile([128, NKT, HPC * HD], dt.float16, tag="qw")
        kw_sb = const.tile([128, NKT, HD], dt.float16, tag="kw")
        vw_sb = const.tile([128, NKT, HD], dt.float16, tag="vw")
        for k4 in range(4):
            nc.sync.dma_start(
                out=kw_sb[:, 4 * k4:4 * (k4 + 1), :],
                in_=kw.ap()[512 * k4:512 * (k4 + 1), :]
                .rearrange("(k p) m -> p k m", p=128))
        for k4 in range(4):
            nc.sync.dma_start(
                out=vw_sb[:, 4 * k4:4 * (k4 + 1), :],
                in_=vw.ap()[512 * k4:512 * (k4 + 1), :]
                .rearrange("(k p) m -> p k m", p=128))
        xs0_global = []
        for k in range(NKT):
            xt = xpool.tile([128, SQC], dt.float16, tag="xt", name="xt")
            nc.gpsimd.dma_start(out=xt, in_=xT.ap()[128 * k:128 * (k + 1), 0:SQC])
            xs0_global.append(xt)
            eng = nc.sync if k < 6 else nc.gpsimd
            eng.dma_start(out=qw_sb[:, k, :], in_=qw.ap()[128 * k:128 * (k + 1), :])
        # o_proj weights are persistent across bodies (true constants)
        ow_sb = const.tile([128, 4, H], dt.float16, tag="ow")
        nc.sync.dma_start(
            out=ow_sb, in_=ow.ap().rearrange("(k p) m -> p k m", p=128))

        qb_sb = const.tile([128, HPC], dt.float32, tag="qb")
        nc.sync.dma_start(out=qb_sb, in_=qb.ap().rearrange("(m p) -> p m", p=128))
        kb_sb = const.tile([128, 1], dt.float32, tag="kb")
        nc.sync.dma_start(out=kb_sb, in_=kb.ap().rearrange("(m p) -> p m", p=128))

        vb_ap = vb.ap()
        vb_bc = const.tile([128, 4, HD], dt.float32, tag="vbb")
        nc.sync.dma_start(
            out=vb_bc,
            in_=bass.AP(tensor=vb_ap.tensor, offset=vb_ap.offset,
                        ap=[[0, 128], [0, 4]] + list(vb_ap.ap)),
        )
        tri_sb = const.tile([128, 128], dt.float16, tag="tri")
        nc.sync.dma_start(out=tri_sb, in_=tri.ap())
        ones_sb = const.tile([128, 128], dt.float16, tag="ones")
        nc.vector.memset(ones_sb, 1.0)

        # persistent activations; v double-buffered across bodies (the
        # carried attention tail of body k reads v while body k+1 projects)
        qT_sb = [const.tile([128, S], dt.float16, tag=f"qT{h}", name=f"qT{h}")
                 for h in range(HPC)]
        kT_sb = const.tile([128, S], dt.float16, tag="kT")
        v_bufs = [const.tile([128, S], dt.float16, tag=f"v{i}", name=f"v{i}")
                  for i in range(2)]

        def _emit_body(first, last, carry, rep):
            vbuf = v_bufs[rep % 2]
            deferred = carry.get("tail")        # fillers of the previous unit
            prev_attn = carry.get("attn3")      # chunk-3 attn of prev body
            xs = carry.get("xs0")               # prefetched chunk-0 x tiles
            attn_by_chunk = {}
            new_carry = {}
            xs_next = []

            def xdma(k, n_chunk, dst):
                def go():
                    xt = xpool.tile([128, SQC], dt.float16, tag="xt", name="xt")
                    eng = nc.sync if k % 2 == 0 else nc.gpsimd
                    eng.dma_start(
                        out=xt,
                        in_=xT.ap()[128 * k:128 * (k + 1),
                                    SQC * n_chunk:SQC * (n_chunk + 1)])
                    dst.append(xt)
                return go

            def o_pair_fillers(nq, m0, at):
                # o_proj for m-blocks m0, m0+1 of chunk nq: 8 matmul fillers
                # + one copy/DMA filler. PE h-order 0..3 so the group start
                # lands on attn[0].
                cell = [None]
                fills = []
                for t in range(2):
                    for hh in range(HPC):
                        def mm(t=t, hh=hh):
                            if cell[0] is None:
                                cell[0] = psum_o.tile([128, 1024], dt.float32,
                                                      tag="po", name="ps_o")
                            m = m0 + t
                            nc.tensor.matmul(
                                cell[0][:, 512 * t:512 * (t + 1)],
                                lhsT=ow_sb[:, hh, 128 * m:128 * (m + 1)],
                                rhs=at[hh], start=(hh == 0), stop=(hh == 3))
                        fills.append(mm)

                def copydma():
                    o_sb = small.tile([128, 1024], dt.float16, tag="osb")
                    if (m0 // 2) % 2 == 0:
                        nc.scalar.copy(out=o_sb, in_=cell[0])
                    else:
                        nc.vector.tensor_copy(out=o_sb, in_=cell[0])
                    for t in range(2):
                        m = m0 + t
                        eng = nc.sync if t == 0 else nc.gpsimd
                        eng.dma_start(
                            out=outp.ap()[128 * m:128 * (m + 1),
                                          SQC * nq:SQC * (nq + 1)],
                            in_=o_sb[:, 512 * t:512 * (t + 1)])
                fills.append(copydma)
                return fills

            def make_unit(n, h):
                """Emit nothing; return (score_items, tail_fillers)."""
                J = 4 * n + 4
                acc2 = acc_pool.tile([128, 1024], dt.float16, tag="acc",
                                     name="acc")
                entries = []          # (j, av_col, rhs_ap) for the AV pass
                state = {"accinit": False}
                items = []

                def acc_op(dst_ap, src_ap, full_pair=False):
                    if not state["accinit"]:
                        # first op covers acc2[:, 0:1024] (full pair) or
                        # acc2[:, 0:512] (n==0 diagonal d0)
                        nc.vector.tensor_copy(out=dst_ap, in_=src_ap)
                        state["accinit"] = True
                    else:
                        nc.vector.tensor_add(out=dst_ap, in0=dst_ap, in1=src_ap)

                qTh = qT_sb[h]
                qbase = SQC * n

                for p in range(2 * n):
                    def item(p=p):
                        ps = psum_sc.tile([128, 1024], dt.float32, tag="sc",
                                          name="ps_s")
                        for t in range(2):
                            j = 2 * p + t
                            nc.tensor.matmul(
                                ps[:, 512 * t:512 * (t + 1)],
                                lhsT=kT_sb[:, 128 * j:128 * (j + 1)],
                                rhs=qTh[:, qbase:qbase + SQC],
                                start=True, stop=True)
                        pt = probs_pool.tile([128, 1024], dt.float16, tag="pt",
                                             name="pt")
                        nc.scalar.activation(out=pt, in_=ps, func=AF.Exp)
                        acc_op(acc2, pt, full_pair=True)
                        entries.append((2 * p, 0, pt[:, 0:512]))
                        entries.append((2 * p + 1, 0, pt[:, 512:1024]))
                    items.append(item)

                def diagA():
                    # blocks d0 (cols 0:512) + d1 (cols 128:512), packed
                    # contiguously: pt[0:512] = d0, pt[512:896] = d1
                    ps = psum_sc.tile([128, 1024], dt.float32, tag="sc",
                                      name="ps_s")
                    nc.tensor.matmul(ps[:, 0:512],
                                     lhsT=kT_sb[:, 128 * 4 * n:128 * (4 * n + 1)],
                                     rhs=qTh[:, qbase:qbase + SQC],
                                     start=True, stop=True)
                    nc.tensor.matmul(ps[:, 512:896],
                                     lhsT=kT_sb[:, 128 * (4 * n + 1):128 * (4 * n + 2)],
                                     rhs=qTh[:, qbase + 128:qbase + SQC],
                                     start=True, stop=True)
                    pt = probs_pool.tile([128, 1024], dt.float16, tag="pt",
                                         name="pt")
                    nc.scalar.activation(out=pt[:, 0:896], in_=ps[:, 0:896],
                                         func=AF.Exp)
                    nc.vector.tensor_mul(out=pt[:, 0:128], in0=pt[:, 0:128],
                                         in1=tri_sb)
                    nc.vector.tensor_mul(out=pt[:, 512:640], in0=pt[:, 512:640],
                                         in1=tri_sb)
                    acc_op(acc2[:, 0:512], pt[:, 0:512])
                    nc.vector.tensor_add(out=acc2[:, 128:512],
                                         in0=acc2[:, 128:512], in1=pt[:, 512:896])
                    entries.append((4 * n, 0, pt[:, 0:512]))
                    entries.append((4 * n + 1, 128, pt[:, 512:896]))

                def diagB():
                    # blocks d2 (cols 256:512) + d3 (cols 384:512):
                    # pt[0:256] = d2, pt[256:384] = d3
                    ps = psum_sc.tile([128, 1024], dt.float32, tag="sc",
                                      name="ps_s")
                    nc.tensor.matmul(ps[:, 0:256],
                                     lhsT=kT_sb[:, 128 * (4 * n + 2):128 * (4 * n + 3)],
                                     rhs=qTh[:, qbase + 256:qbase + SQC],
                                     start=True, stop=True)
                    nc.tensor.matmul(ps[:, 256:384],
                                     lhsT=kT_sb[:, 128 * (4 * n + 3):128 * (4 * n + 4)],
                                     rhs=qTh[:, qbase + 384:qbase + SQC],
                                     start=True, stop=True)
                    pt = probs_pool.tile([128, 1024], dt.float16, tag="pt",
                                         name="pt")
                    nc.scalar.activation(out=pt[:, 0:384], in_=ps[:, 0:384],
                                         func=AF.Exp)
                    nc.vector.tensor_mul(out=pt[:, 0:128], in0=pt[:, 0:128],
                                         in1=tri_sb)
                    nc.vector.tensor_mul(out=pt[:, 256:384], in0=pt[:, 256:384],
                                         in1=tri_sb)
                    nc.vector.tensor_add(out=acc2[:, 256:512],
                                         in0=acc2[:, 256:512], in1=pt[:, 0:256])
                    nc.vector.tensor_add(out=acc2[:, 384:512],
                                         in0=acc2[:, 384:512], in1=pt[:, 256:384])
                    entries.append((4 * n + 2, 256, pt[:, 0:256]))
                    entries.append((4 * n + 3, 384, pt[:, 256:384]))

                items.append(diagA)
                items.append(diagB)

                def make_tail():
                    # AV + den matmul fillers + final recip/normalize
                    cell = [None]
                    fills = []
                    for j, c, rhs in entries:
                        def mm(j=j, c=c, rhs=rhs):
                            if cell[0] is None:
                                cell[0] = psum_tl.tile([128, 1024], dt.float32,
                                                       tag="tl", name="ps_tl")
                            nc.tensor.matmul(cell[0][:, c:512],
                                             lhsT=vbuf[:, 128 * j:128 * (j + 1)],
                                             rhs=rhs,
                                             start=(j == 0), stop=(j == J - 1))
                        fills.append(mm)

                    def den():
                        two = n > 0
                        nc.tensor.matmul(cell[0][:, 512:1024], lhsT=ones_sb,
                                         rhs=acc2[:, 0:512],
                                         start=True, stop=not two)
                        if two:
                            nc.tensor.matmul(cell[0][:, 512:1024], lhsT=ones_sb,
                                             rhs=acc2[:, 512:1024],
                                             start=False, stop=True)
                    fills.append(den)

                    def recipnorm():
                        recip = small.tile([128, SQC], dt.float32, tag="recip",
                                           name="recip")
                        nc.vector.reciprocal_approx_fast(out=recip,
                                                         in_=cell[0][:, 512:1024])
                        at = attn_pool.tile([128, SQC], dt.float16, tag="at",
                                            name="at")
                        nc.vector.tensor_mul(out=at, in0=cell[0][:, 0:512],
                                             in1=recip)
                        attn_by_chunk.setdefault(n, []).append(at)
                    fills.append(recipnorm)
                    return fills

                return items, make_tail

            def interleave(items, fillers):
                ni = len(items)
                done = 0
                for i, it in enumerate(items):
                    it()
                    target = (i + 1) * len(fillers) // ni
                    while done < target:
                        fillers[done]()
                        done += 1

            for n in range(NQC):
                # ---------------- projections for chunk n ----------------
                if xs is None:
                    xs = []
                    for k in range(NKT):
                        xdma(k, 0, xs)()
                # kT + v share one score-pair PSUM slot
                ps_kv = psum_sc.tile([128, 1024], dt.float32, tag="sc")
                for k in range(NKT):
                    nc.tensor.matmul(ps_kv[:, 0:512], lhsT=kw_sb[:, k, :],
                                     rhs=xs[k], start=(k == 0),
                                     stop=(k == NKT - 1))
                for jj in range(4):
                    for k in range(NKT):
                        nc.tensor.matmul(
                            ps_kv[:, 512 + 128 * jj:512 + 128 * (jj + 1)],
                            lhsT=xs[k][:, 128 * jj:128 * (jj + 1)],
                            rhs=vw_sb[:, k, :],
                            start=(k == 0), stop=(k == NKT - 1))
                nc.vector.tensor_scalar_add(out=kT_sb[:, SQC * n:SQC * (n + 1)],
                                            in0=ps_kv[:, 0:512],
                                            scalar1=kb_sb[:, 0:1])
                for jj in range(4):
                    nc.vector.tensor_add(
                        out=vbuf[:, SQC * n + 128 * jj:SQC * n + 128 * (jj + 1)],
                        in0=ps_kv[:, 512 + 128 * jj:512 + 128 * (jj + 1)],
                        in1=vb_bc[:, jj, :])
                for hp in range(2):
                    ps_q = psum_sc.tile([128, 1024], dt.float32, tag="sc")
                    for t in range(2):
                        hh = 2 * hp + t
                        for k in range(NKT):
                            nc.tensor.matmul(
                                ps_q[:, 512 * t:512 * (t + 1)],
                                lhsT=qw_sb[:, k, 128 * hh:128 * (hh + 1)],
                                rhs=xs[k], start=(k == 0), stop=(k == NKT - 1))
                    for t in range(2):
                        hh = 2 * hp + t
                        nc.vector.tensor_scalar_add(
                            out=qT_sb[hh][:, SQC * n:SQC * (n + 1)],
                            in0=ps_q[:, 512 * t:512 * (t + 1)],
                            scalar1=qb_sb[:, hh:hh + 1])
                xs = None

                # ---------------- attention for chunk n ----------------
                # x prefetch for the next chunk (or next body's chunk 0) is
                # spread across the units as fillers.
                pref_chunk = n + 1 if n < NQC - 1 else (0 if not last else None)
                pref_dst = xs_next if n < NQC - 1 else \
                    new_carry.setdefault("xs0", [])
                o_src = attn_by_chunk.get(n - 1) if n >= 1 else prev_attn
                o_nq = n - 1 if n >= 1 else NQC - 1

                for h in range(HPC):
                    items, make_tail = make_unit(n, h)
                    fill = []
                    opairs = None
                    if o_src is not None:
                        opairs = (o_pair_fillers(o_nq, 8 * h // 2, o_src),
                                  o_pair_fillers(o_nq, 8 * h // 2 + 2, o_src))
                    if h == 0:
                        if deferred is not None:
                            fill += deferred
                        if opairs is not None:
                            fill += opairs[0] + opairs[1]
                    else:
                        if opairs is not None:
                            fill += opairs[0]
                        if deferred is not None:
                            fill += deferred
                        if opairs is not None:
                            fill += opairs[1]
                    if pref_chunk is not None:
                        fill += [xdma(4 * h + i, pref_chunk, pref_dst)
                                 for i in range(4)]
                    interleave(items, fill)
                    deferred = make_tail()

                if n < NQC - 1:
                    xs = xs_next or None   # filled by the xdma fillers, k order
                    xs_next = []

            if last:
                for f in deferred:
                    f()
                at3 = attn_by_chunk[NQC - 1]
                for mp in range(8):
                    for f in o_pair_fillers(NQC - 1, 2 * mp, at3):
                        f()
                return {}
            new_carry["tail"] = deferred
            new_carry["attn3"] = attn_by_chunk[NQC - 1]
            return new_carry

        carry = {"xs0": xs0_global}
        for rep in range(repeat):
            carry = _emit_body(first=(rep == 0), last=(rep == repeat - 1),
                               carry=carry, rep=rep)

    nc.compile()
    return nc


def _get_nc(repeat=1):
    key = f"nc{repeat}"
    if key not in _CACHE:
        _CACHE[key] = _build_program(repeat)
    return _CACHE[key]


def _make_in_maps(hidden_states, attention_mask, q_w, q_b, kl_w, kl_b, vl_w, vl_b,
                  kr_w, kr_b, vr_w, vr_b, o_w):
    scale = 1.0 / math.sqrt(HD)
    tri01 = (np.asarray(attention_mask[0, 0, :128, :128]) == 0).T.astype(FP16)
    kr_f = np.asarray(kr_w, np.float32)
    vr_f = np.asarray(vr_w, np.float32)
    in_maps = []
    for c in range(NCORES):
        b, g = divmod(c, NLH)
        sl = slice(LD * g, LD * (g + 1))
        xTc = np.ascontiguousarray(np.asarray(hidden_states[b], np.float32).T
                                   ).astype(FP16)
        # merged latent->head weights: k = x @ (kr_w @ kl_w)^T + (kr_w@kl_b + kr_b)
        kw_eff = kr_f @ np.asarray(kl_w[sl], np.float32)
        vw_eff = vr_f @ np.asarray(vl_w[sl], np.float32)
        kb_eff = kr_f @ np.asarray(kl_b[sl], np.float32) + np.asarray(kr_b, np.float32)
        vb_eff = vr_f @ np.asarray(vl_b[sl], np.float32) + np.asarray(vr_b, np.float32)
        in_maps.append({
            "xT": xTc,
            "qwT": np.ascontiguousarray(
                (np.asarray(q_w[sl], np.float32) * scale).T).astype(FP16),
            "kwT": np.ascontiguousarray(kw_eff.T).astype(FP16),
            "vwT": np.ascontiguousarray(vw_eff.T).astype(FP16),
            "owT": np.ascontiguousarray(np.asarray(o_w, np.float32)[:, sl].T
                                        ).astype(FP16),
            "qb": (np.asarray(q_b[sl], np.float32) * scale),
            "kb": kb_eff,
            "vb": vb_eff,
            "tri": tri01,
        })
    return in_maps


def _gather(results, o_b):
    o_b = np.asarray(o_b, np.float32)
    outs = []
    for b in range(B):
        acc = np.zeros((H, S), np.float32)
        for g in range(NLH):
            acc += results[b * NLH + g]["out"].astype(np.float32)
        outs.append(acc.T + o_b[None, :])
    return np.stack(outs).astype(np.float32)


def kernel(hidden_states, position_ids, attention_mask, q_w, q_b, kl_w, kl_b,
           vl_w, vl_b, kr_w, kr_b, vr_w, vr_b, o_w, o_b):
    from concourse.bass_utils import run_bass_kernel_spmd

    nc = _get_nc()
    in_maps = _make_in_maps(hidden_states, attention_mask, q_w, q_b, kl_w, kl_b,
                            vl_w, vl_b, kr_w, kr_b, vr_w, vr_b, o_w)
    res = run_bass_kernel_spmd(nc, in_maps, core_ids=list(range(NCORES)))
    return _gather(res.results, o_b)
